# revision 18
# baseline (speedup 1.0000x reference)
"""Trainium2 Bass kernel for nn_AdaptiveAutoCorrelation (8-core data-parallel).

v4 — from the 253us v3:
  * q/k staged to HBM as fp16 host-side (half the input DMA, no cast op);
    LN runs all-fp16 (reduces emit fp16 stats under allow_low_precision,
    normalize is all-fp16 TT) to hit DVE 2x modes.
  * Output staged fp16 (host casts back to f32).
  * Orphan (Nyquist-only) f-tiles computed by a (-1)^p signed reduce +
    1-column matmul instead of two full DFT chains (-18k PE cycles).
  * Delay aggregation split: group 0 on DVE (fp16 stt MAC), group 1 on
    the PE (nw_k*I stationaries) so the 50%-throttled PE isn't the tail
    bottleneck.
"""
import math

import numpy as np

L = 1536
H, E = 8, 64
R = H * E  # 512
B = 8
NT = L // 128  # 12 l-tiles
SCALES = [1, 2, 4]
KT = [12, 6, 3]  # contraction tiles per scale (pooled-first)
FBINS = [L // s // 2 + 1 for s in SCALES]  # [769, 385, 193]
FT = [(f + 127) // 128 for f in FBINS]  # f-tiles per re/im block: [7, 4, 2]
NFT = 2 * sum(FT)  # 26 total f-tiles
TOPK = int(math.log(L))  # 7
LN_EPS = 1e-5
GPK = 12  # tiles packed per gather row (12KB rows)
NGRP = NT // GPK  # 2 gathers per delay
NW = 2 * L - 128 * (GPK - 1)  # 2688 rows in the sliding-window gather buffer
MC_SHIFT = 2.0 ** -14  # undo host-side M prescale (exact power of two)
MAG_EPS2 = 1e-6  # |kf|^2 floor: keeps rs finite for zero-padded bins
LN_CHUNKS = [(0, 4), (4, 8), (8, 12)]

# global ftile index bases (for S / M layout): per scale, re tiles then im
_FT_BASE = []
_acc = 0
for _s in range(len(SCALES)):
    _FT_BASE.append((_acc, _acc + FT[_s]))
    _acc += 2 * FT[_s]

_CACHE = {}


def _build_constants(scale_weights, frequency_filter):
    """D chains per scale [2*nf,128,nkt*128] fp16, M [NFT,128,L] fp16."""
    f_sig = 1.0 / (1.0 + np.exp(-np.float64(frequency_filter[0])))
    sw = np.asarray(scale_weights[: len(SCALES)], np.float64)
    w = np.exp(sw - sw.max())
    w = w / w.sum()

    d_chains = []
    M = np.zeros((NFT * 128, L), np.float64)
    for si, s in enumerate(SCALES):
        Ls = L // s
        F = FBINS[si]
        nf = FT[si]
        nkt = KT[si]
        t = np.arange(Ls)[:, None]
        f = np.arange(F)[None, :]
        ang = 2.0 * np.pi * t * f / Ls
        Dre = np.zeros((Ls, nf * 128))
        Dim = np.zeros((Ls, nf * 128))
        Dre[:, :F] = np.cos(ang)
        Dim[:, :F] = -np.sin(ang)
        # chain layout: [lf, p, kt*128 + fc] = blk[kt*128 + p, j*128 + fc]
        cr = Dre.reshape(nkt, 128, nf, 128).transpose(2, 1, 0, 3)
        ci = Dim.reshape(nkt, 128, nf, 128).transpose(2, 1, 0, 3)
        ch = np.concatenate([cr, ci], axis=0).reshape(2 * nf, 128, nkt * 128)
        d_chains.append(np.ascontiguousarray(ch.astype(np.float16)))

        reb, imb = _FT_BASE[si]
        tt = np.arange(Ls)[None, :]
        cf = np.where((f.T == 0) | (f.T == F - 1), 1.0, 2.0)
        ang2 = 2.0 * np.pi * f.T * tt / Ls
        Mre = cf * np.cos(ang2) / Ls  # [F, Ls]
        Mim = -cf * np.sin(ang2) / Ls
        if Ls != L:
            P = np.zeros((Ls, L))
            co = np.clip((np.arange(L) + 0.5) * (Ls / L) - 0.5, 0, Ls - 1)
            lo = np.floor(co).astype(int)
            hi = np.minimum(lo + 1, Ls - 1)
            fr = co - lo
            P[lo, np.arange(L)] += 1 - fr
            P[hi, np.arange(L)] += fr
            Mre = Mre @ P
            Mim = Mim @ P
        scale = w[si] * f_sig / R * 16384.0  # 2^14 prescale for fp16 range
        M[reb * 128 : reb * 128 + F] = Mre * scale
        M[imb * 128 : imb * 128 + F] = Mim * scale

    M_t = np.ascontiguousarray(M.reshape(NFT, 128, L).astype(np.float16))
    # pool-by-2 packing matrices: P2a -> out cols [0,64), P2b -> [64,128)
    P2 = np.zeros((2, 128, 128), np.float16)
    for t_ in range(128):
        P2[0, t_, t_ // 2] = 0.5
        P2[1, t_, 64 + t_ // 2] = 0.5
    I128 = np.eye(128, dtype=np.float16)
    sgn = ((-1.0) ** np.arange(128)).astype(np.float16).reshape(128, 1)
    return d_chains, M_t, P2, I128, sgn


def _build_graph():
    import concourse.bacc as bacc
    import concourse.bass as bass
    import concourse.mybir as mybir
    import concourse.tile as tile

    AF = mybir.ActivationFunctionType
    OP = mybir.AluOpType
    f32 = mybir.dt.float32
    f16 = mybir.dt.float16
    u32 = mybir.dt.uint32

    nc = bacc.Bacc("TRN2", debug=False)
    q_d = nc.dram_tensor("q", [NT, 128, R], f16, kind="ExternalInput")
    k_d = nc.dram_tensor("k", [NT, 128, R], f16, kind="ExternalInput")
    vw_d = nc.dram_tensor("vw", [NW, GPK * R], f16, kind="ExternalInput")
    d_ds = [
        nc.dram_tensor(
            f"dmat{si}", [2 * FT[si], 128, KT[si] * 128], f16,
            kind="ExternalInput",
        )
        for si in range(len(SCALES))
    ]
    m_d = nc.dram_tensor("mmat", [NFT, 128, L], f16, kind="ExternalInput")
    p_d = nc.dram_tensor("pmat", [2, 128, 128], f16, kind="ExternalInput")
    i_d = nc.dram_tensor("imat", [128, 128], f16, kind="ExternalInput")
    g_d = nc.dram_tensor("sgn", [128, 1], f16, kind="ExternalInput")
    o_d = nc.dram_tensor("out", [NT, 128, R], f16, kind="ExternalOutput")

    with tile.TileContext(nc) as tc:
        with (
            tc.tile_pool(name="qk", bufs=1) as qkpool,
            tc.tile_pool(name="small", bufs=1) as spool,
            tc.tile_pool(name="lnraw", bufs=3) as rpool,
            tc.tile_pool(name="lnstat", bufs=2) as stpool,
        ):
            xn = {}
            for name in ("q", "k"):
                xn[(name, 0)] = qkpool.tile(
                    [128, NT, R], f16, tag=f"{name}mega", name=f"{name}mega"
                )
                for si, nkt in ((1, 6), (2, 3)):
                    xn[(name, si)] = qkpool.tile(
                        [128, nkt, R], f16, tag=f"{name}p{si}",
                        name=f"{name}p{si}",
                    )

            # raw chunk loads first so transfers start ASAP
            raws = {}
            for c0, c1 in LN_CHUNKS:
                for name, src in (("q", q_d), ("k", k_d)):
                    raw = rpool.tile([128, c1 - c0, R], f16, tag=f"raw{name}")
                    nc.sync.dma_start(
                        raw[:], src.ap()[c0:c1].rearrange("t p r -> p t r")
                    )
                    raws[(name, c0)] = raw

            eps_ln = spool.tile([128, 1], f32, tag="eps_ln")
            nc.vector.memset(eps_ln[:], LN_EPS)
            eps_mag = spool.tile([128, 1], f32, tag="eps_mag")
            nc.vector.memset(eps_mag[:], MAG_EPS2)
            p2a = spool.tile([128, 128], f16, tag="p2a")
            p2b = spool.tile([128, 128], f16, tag="p2b")
            imat = spool.tile([128, 128], f16, tag="imat")
            sgn = spool.tile([128, 1], f16, tag="sgn")
            nc.sync.dma_start(p2a[:], p_d.ap()[0])
            nc.sync.dma_start(p2b[:], p_d.ap()[1])
            nc.sync.dma_start(imat[:], i_d.ap())
            nc.sync.dma_start(sgn[:], g_d.ap())
            iota2 = spool.tile([128, NGRP], u32, tag="iota2")
            for g in range(NGRP):
                nc.gpsimd.iota(
                    iota2[:, g : g + 1], pattern=[[0, 1]],
                    base=128 * GPK * g, channel_multiplier=1,
                )

            # ---- chunked pipelined layernorm (all fp16) ----
            for c0, c1 in LN_CHUNKS:
                ntc = c1 - c0
                A = ntc * H
                for name in ("q", "k"):
                    raw = raws[(name, c0)]
                    x3 = raw[:].rearrange("p t (h e) -> p (t h) e", e=E)
                    sq = rpool.tile([128, ntc, R], f16, tag=f"sq{name}")
                    nc.scalar.activation(sq[:], raw[:], AF.Square)
                    st1 = stpool.tile([128, A], f16, tag=f"st1{name}")
                    st2 = stpool.tile([128, A], f16, tag=f"st2{name}")
                    with nc.allow_low_precision("LN stats fp16 ok"):
                        nc.vector.tensor_reduce(
                            st1[:], x3, mybir.AxisListType.X, OP.add
                        )
                        nc.vector.tensor_reduce(
                            st2[:],
                            sq[:].rearrange("p t (h e) -> p (t h) e", e=E),
                            mybir.AxisListType.X, OP.add,
                        )
                    mean = stpool.tile([128, A], f32, tag=f"mn{name}")
                    nc.vector.tensor_scalar_mul(mean[:], st1[:], 1.0 / E)
                    m2 = stpool.tile([128, A], f32, tag=f"m2{name}")
                    nc.vector.tensor_mul(m2[:], mean[:], mean[:])
                    var = stpool.tile([128, A], f32, tag=f"vr{name}")
                    nc.vector.scalar_tensor_tensor(
                        var[:], st2[:], 1.0 / E, m2[:],
                        op0=OP.mult, op1=OP.subtract,
                    )
                    std = stpool.tile([128, A], f32, tag=f"sd{name}")
                    nc.scalar.activation(
                        std[:], var[:], AF.Sqrt, bias=eps_ln[:]
                    )
                    rstd = stpool.tile([128, A], f32, tag=f"rs{name}")
                    nc.vector.reciprocal(rstd[:], std[:])
                    rs16 = stpool.tile([128, A], f16, tag=f"rsh{name}")
                    nc.vector.tensor_copy(rs16[:], rstd[:])
                    sr16 = stpool.tile([128, A], f16, tag=f"srh{name}")
                    nc.vector.tensor_mul(sr16[:], mean[:], rstd[:])
                    # xhat = x*rstd - mean*rstd  (all-fp16, in-place 2nd op)
                    eng = nc.vector if name == "q" else nc.gpsimd
                    mg3 = xn[(name, 0)][:, c0:c1, :].rearrange(
                        "p t (h e) -> p (t h) e", e=E
                    )
                    rs3 = rs16[:].rearrange("p (a o) -> p a o", o=1)
                    x3b, rs_b = bass.broadcast_tensor_aps(x3, rs3)
                    eng.tensor_tensor(mg3, x3b, rs_b, OP.mult)
                    sr3 = sr16[:].rearrange("p (a o) -> p a o", o=1)
                    mg3b, sr_b = bass.broadcast_tensor_aps(mg3, sr3)
                    eng.tensor_tensor(mg3, mg3b, sr_b, OP.subtract)

            # ---- DFT + spectral + lagged irfft (mean_corr) ----
            S16 = spool.tile([128, 32], f16, tag="s16")
            nc.vector.memset(S16[:], 0.0)
            with (
                tc.tile_pool(name="psum", bufs=5, space="PSUM") as pp,
                tc.tile_pool(name="mcpsum", bufs=1, space="PSUM") as mcp,
                tc.tile_pool(name="dstream", bufs=4) as dpool,
                tc.tile_pool(name="mstream", bufs=2) as mpool,
                tc.tile_pool(name="spec", bufs=2) as scp,
            ):
                mc_ps = [
                    mcp.tile([1, 512], f32, tag=f"mc{nt}", name=f"mc{nt}")
                    for nt in range(3)
                ]

                # pools first: the PE runs them inside the LN-wait window
                for name in ("q", "k"):
                    for si, nkt in ((1, 6), (2, 3)):
                        srcm = xn[(name, si - 1)]
                        dst = xn[(name, si)]
                        for j2 in range(nkt):
                            ps = pp.tile(
                                [128, 512], f32, tag="dftps", name="poolps"
                            )
                            nc.tensor.matmul(
                                ps[:], p2a[:], srcm[:, 2 * j2, :],
                                start=True, stop=False,
                            )
                            nc.tensor.matmul(
                                ps[:], p2b[:], srcm[:, 2 * j2 + 1, :],
                                start=False, stop=True,
                            )
                            nc.scalar.activation(dst[:, j2, :], ps[:], AF.Copy)

                def is_orphan(si2, j2):
                    # last f-tile of scales 1,2 holds a single (Nyquist) bin
                    # whose imaginary part is exactly zero
                    return si2 < 2 and j2 == FT[si2] - 1

                # big scale-1 pair last: its PE chains hide the mc tail
                pair_order = (
                    [(0, j) for j in (0, 1, 2, 3, 4, 6)]
                    + [(1, j) for j in range(FT[1])]
                    + [(2, j) for j in range(FT[2])]
                    + [(0, 5)]
                )
                pair_list = []
                for si, j in pair_order:
                    reb, imb = _FT_BASE[si]
                    pair_list.append((si, j, reb + j, imb + j))
                n_pairs = len(pair_list)

                def emit_mc(pi2, first_mm):
                    si2, j2, ftr2, fti2 = pair_list[pi2]
                    fts = (ftr2,) if is_orphan(si2, j2) else (ftr2, fti2)
                    for ft in fts:
                        mtile = mpool.tile([128, L], f16, tag="mtile")
                        nc.sync.dma_start(mtile[:], m_d.ap()[ft])
                        for nt in range(3):
                            nc.tensor.matmul(
                                mc_ps[nt][:], S16[:, ft : ft + 1],
                                mtile[:, nt * 512 : (nt + 1) * 512],
                                start=first_mm,
                                stop=(
                                    pi2 == n_pairs - 1 and ft == fts[-1]
                                    and nt == 2
                                ),
                                skip_group_check=True,
                            )
                        first_mm = False
                    return first_mm

                def emit_orphan(si, ftr):
                    # single Nyquist bin: X[Ny] = sum_p sgn[p] sum_t x[p,t,:]
                    xq = xn[("q", si)]
                    xk = xn[("k", si)]
                    nkt = KT[si]
                    ys = {}
                    for nm, xm in (("q", xq), ("k", xk)):
                        y = scp.tile([128, 512], f16, tag=f"oy{nm}")
                        with nc.allow_low_precision("orphan fold fp16"):
                            nc.vector.tensor_reduce(
                                y[:],
                                xm[:].rearrange("p t r -> p r t"),
                                mybir.AxisListType.X, OP.add,
                            )
                        ps = pp.tile([128, 512], f32, tag="dftps", name="orps")
                        nc.tensor.matmul(
                            ps[0:1, :], sgn[:], y[:], start=True, stop=True
                        )
                        sb = scp.tile([1, 512], f32, tag=f"oyS{nm}")
                        nc.scalar.activation(sb[:], ps[0:1, :], AF.Copy)
                        ys[nm] = sb
                    mag = scp.tile([1, 512], f32, tag="omag")
                    nc.vector.scalar_tensor_tensor(
                        mag[:], ys["k"][:], 0.0, ys["k"][:], op0=OP.bypass,
                        op1=OP.mult,
                    )
                    smag = scp.tile([1, 512], f32, tag="osmag")
                    nc.scalar.activation(
                        smag[:], mag[:], AF.Sqrt, bias=eps_mag[0:1, 0:1]
                    )
                    rs = scp.tile([1, 512], f32, tag="ors")
                    nc.vector.reciprocal_approx_fast(rs[:], smag[:])
                    khr = scp.tile([1, 512], f32, tag="okhr")
                    nc.vector.tensor_mul(khr[:], ys["k"][:], rs[:])
                    scr = scp.tile([1, 512], f32, tag="oscr")
                    a1 = scp.tile([1, 1], f32, tag="oa1")
                    nc.vector.scalar_tensor_tensor(
                        scr[:], ys["q"][:], 0.0, khr[:], op0=OP.bypass,
                        op1=OP.mult, accum_out=a1[:],
                    )
                    nc.vector.tensor_copy(S16[0:1, ftr : ftr + 1], a1[:])

                MC_LAG = 2
                first_mm = True
                for pi, (si, j, ftr, fti) in enumerate(pair_list):
                    nkt = KT[si]
                    qx = xn[("q", si)]
                    kx = xn[("k", si)]
                    if is_orphan(si, j):
                        emit_orphan(si, ftr)
                        if pi >= MC_LAG:
                            first_mm = emit_mc(pi - MC_LAG, first_mm)
                        continue
                    psl = {}
                    # load each D tile once; q chain then k chain share it
                    for li, lf in enumerate((j, FT[si] + j)):
                        part = "re" if li == 0 else "im"
                        dch = dpool.tile([128, nkt, 128], f16, tag=f"d{si}")
                        nc.sync.dma_start(
                            dch[:].rearrange("p a b -> p (a b)"),
                            d_ds[si].ap()[lf],
                        )
                        psq = pp.tile(
                            [128, 512], f32, tag="dftps", name="psq"
                        )
                        psk = pp.tile(
                            [128, 512], f32, tag="dftps", name="psk"
                        )
                        for kt in range(nkt):
                            nc.tensor.matmul(
                                psq[:], dch[:, kt, :], qx[:, kt, :],
                                start=(kt == 0), stop=(kt == nkt - 1),
                            )
                            nc.tensor.matmul(
                                psk[:], dch[:, kt, :], kx[:, kt, :],
                                start=(kt == 0), stop=(kt == nkt - 1),
                            )
                        for nm, ps in ((f"q{part}", psq), (f"k{part}", psk)):
                            sb = scp.tile([128, 512], f16, tag=f"{nm}S")
                            nc.scalar.activation(sb[:], ps[:], AF.Copy)
                            psl[nm] = sb
                    # mc matmuls, lagged so the PE never waits on spectral
                    if pi >= MC_LAG:
                        first_mm = emit_mc(pi - MC_LAG, first_mm)
                    qreS, kreS = psl["qre"], psl["kre"]
                    qimS, kimS = psl["qim"], psl["kim"]
                    sq1 = scp.tile([128, 512], f16, tag="sq1")
                    nc.scalar.activation(sq1[:], kreS[:], AF.Square)
                    sq2 = scp.tile([128, 512], f16, tag="sq2")
                    nc.vector.scalar_tensor_tensor(
                        sq2[:], kimS[:], 0.0, kimS[:], op0=OP.bypass,
                        op1=OP.mult,
                    )
                    mag2 = scp.tile([128, 512], f16, tag="mag2")
                    nc.vector.tensor_add(mag2[:], sq1[:], sq2[:])
                    mag = scp.tile([128, 512], f32, tag="mag")
                    nc.scalar.activation(
                        mag[:], mag2[:], AF.Sqrt, bias=eps_mag[:, 0:1]
                    )
                    rs = scp.tile([128, 512], f32, tag="rs")
                    nc.vector.reciprocal_approx_fast(rs[:], mag[:])
                    khr = scp.tile([128, 512], f16, tag="khr")
                    khi = scp.tile([128, 512], f16, tag="khi")
                    nc.vector.tensor_mul(khr[:], kreS[:], rs[:])
                    nc.vector.tensor_mul(khi[:], kimS[:], rs[:])
                    scr = scp.tile([128, 512], f16, tag="scr")
                    scr2 = scp.tile([128, 512], f16, tag="scr2")
                    a1 = scp.tile([128, 1], f32, tag="a1")
                    a2 = scp.tile([128, 1], f32, tag="a2")
                    a3 = scp.tile([128, 1], f32, tag="a3")
                    a4 = scp.tile([128, 1], f32, tag="a4")
                    nc.vector.scalar_tensor_tensor(
                        scr[:], qreS[:], 0.0, khr[:], op0=OP.bypass,
                        op1=OP.mult, accum_out=a1[:],
                    )
                    nc.vector.scalar_tensor_tensor(
                        scr2[:], qimS[:], 0.0, khi[:], op0=OP.bypass,
                        op1=OP.mult, accum_out=a2[:],
                    )
                    nc.vector.tensor_add(S16[:, ftr : ftr + 1], a1[:], a2[:])
                    nc.vector.scalar_tensor_tensor(
                        scr[:], qimS[:], 0.0, khr[:], op0=OP.bypass,
                        op1=OP.mult, accum_out=a3[:],
                    )
                    nc.vector.scalar_tensor_tensor(
                        scr2[:], qreS[:], 0.0, khi[:], op0=OP.bypass,
                        op1=OP.mult, accum_out=a4[:],
                    )
                    nc.vector.tensor_sub(S16[:, fti : fti + 1], a3[:], a4[:])
                for pi in range(n_pairs - MC_LAG, n_pairs):
                    first_mm = emit_mc(pi, first_mm)

                mc_row = spool.tile([1, L], f32, tag="mcrow")
                for nt in range(3):
                    nc.vector.tensor_scalar_mul(
                        mc_row[:, nt * 512 : (nt + 1) * 512], mc_ps[nt][:],
                        MC_SHIFT,
                    )

            # ---- top-7 + softmax ----
            mc8 = spool.tile([1, 8], f32, tag="mc8")
            mcidx = spool.tile([1, 8], u32, tag="mcidx")
            nc.vector.max(mc8[:], mc_row[:])
            nc.vector.max_index(mcidx[:], mc8[:], mc_row[:])
            negmax = spool.tile([1, 1], f32, tag="negmax")
            nc.vector.tensor_scalar_mul(negmax[:], mc8[:, 0:1], -1.0)
            e7 = spool.tile([1, TOPK], f32, tag="e7")
            nc.scalar.activation(e7[:], mc8[:, 0:TOPK], AF.Exp, bias=negmax[:])
            ssum = spool.tile([1, 1], f32, tag="ssum")
            nc.vector.tensor_reduce(ssum[:], e7[:], mybir.AxisListType.X, OP.add)
            rsum = spool.tile([1, 1], f32, tag="rsum")
            nc.vector.reciprocal(rsum[:], ssum[:])
            nw = spool.tile([1, TOPK], f32, tag="nw")
            nc.vector.tensor_scalar_mul(nw[:], e7[:], rsum[:, 0:1])
            nw128 = spool.tile([128, TOPK], f32, tag="nw128")
            nc.gpsimd.partition_broadcast(nw128[:], nw[:])
            d128a = spool.tile([128, TOPK], u32, tag="d128a")
            nc.gpsimd.partition_broadcast(d128a[:], mcidx[:, 0:TOPK])
            # all 2*7 gather indices in one DVE op
            idx_all = spool.tile([128, NGRP, TOPK], u32, tag="idxall")
            ii = iota2[:].rearrange("p (g o) -> p g o", o=1)
            dd = d128a[:].rearrange("p (o k) -> p o k", o=1)
            iib, ddb = bass.broadcast_tensor_aps(ii, dd)
            nc.vector.tensor_tensor(idx_all[:], iib, ddb, OP.add)
            # weighted identity stationaries for the PE half of the MAC
            wI = []
            for kk in range(TOPK):
                wt = spool.tile([128, 128], f16, tag=f"wI{kk}", name=f"wI{kk}")
                nc.vector.tensor_scalar_mul(wt[:], imat[:], nw128[:, kk : kk + 1])
                wI.append(wt)

            # ---- gather (1 op per delay) + MAC split DVE/PE per slot ----
            HG = GPK // 2  # l-tiles per engine half
            with (
                tc.tile_pool(name="gather", bufs=3) as gpool,
                tc.tile_pool(name="gpsum", bufs=6, space="PSUM") as gpp,
            ):
                acc = gpool.tile([128, HG, R], f32, tag="acc", bufs=1)
                acc16 = gpool.tile([128, NT, R], f16, tag="acc16", bufs=1)
                gps = [
                    gpp.tile([128, 512], f32, tag="gps", name=f"gps{c}")
                    for c in range(HG)
                ]
                for kk in range(TOPK):
                    slot = gpool.tile([128, GPK * R], f16, tag="slot", bufs=3)
                    nc.gpsimd.indirect_dma_start(
                        out=slot[:],
                        out_offset=None,
                        in_=vw_d.ap(),
                        in_offset=bass.IndirectOffsetOnAxis(
                            ap=idx_all[:, 0, kk : kk + 1], axis=0
                        ),
                    )
                    av = acc[:].rearrange("p t r -> p (t r)")
                    half = slot[:, 0 : HG * R]
                    if kk == 0:
                        nc.vector.tensor_scalar_mul(av, half, nw128[:, 0:1])
                    else:
                        nc.vector.scalar_tensor_tensor(
                            av, half, nw128[:, kk : kk + 1], av,
                            op0=OP.mult, op1=OP.add,
                        )
                    for c in range(HG):
                        nc.tensor.matmul(
                            gps[c][:], wI[kk][:],
                            slot[:, (HG + c) * R : (HG + c + 1) * R],
                            start=(kk == 0), stop=(kk == TOPK - 1),
                        )
                    if kk == TOPK - 1:
                        for c in range(HG):
                            nc.scalar.activation(
                                acc16[:, c, :], acc[:, c, :], AF.Copy
                            )
                            nc.sync.dma_start(o_d.ap()[c], acc16[:, c, :])
                        for c in range(HG):
                            nc.scalar.activation(
                                acc16[:, HG + c, :], gps[c][:], AF.Copy
                            )
                            nc.sync.dma_start(
                                o_d.ap()[HG + c], acc16[:, HG + c, :]
                            )

    nc.compile()
    return nc


def _get_graph():
    if "nc" not in _CACHE:
        _CACHE["nc"] = _build_graph()
    return _CACHE["nc"]


def _make_in_maps(queries, keys, values, scale_weights, frequency_filter):
    d_chains, M_t, P2, I128, sgn = _build_constants(
        np.asarray(scale_weights, np.float64),
        np.asarray(frequency_filter, np.float64),
    )
    q = np.asarray(queries, np.float32).reshape(B, NT, 128, R).astype(np.float16)
    k = np.asarray(keys, np.float32).reshape(B, NT, 128, R).astype(np.float16)
    v = np.asarray(values, np.float32).reshape(B, L, R)
    vv = np.concatenate([v, v], axis=1).astype(np.float16)  # [B, 2L, R]
    # sliding-window buffer: vw[b, i, c, :] = vv[b, i + 128*c, :], c < GPK
    st = vv.strides
    vw = np.lib.stride_tricks.as_strided(
        vv, shape=(B, NW, GPK, R), strides=(st[0], st[1], 128 * st[1], st[2])
    )
    in_maps = []
    for b in range(B):
        m = {
            "q": np.ascontiguousarray(q[b]),
            "k": np.ascontiguousarray(k[b]),
            "vw": np.ascontiguousarray(vw[b]).reshape(NW, GPK * R),
            "mmat": M_t,
        }
        m["pmat"] = P2
        m["imat"] = I128
        m["sgn"] = sgn
        for si in range(len(SCALES)):
            m[f"dmat{si}"] = d_chains[si]
        in_maps.append(m)
    return in_maps


def kernel(queries, keys, values, scale_weights, frequency_filter, attn_mask=None):
    from concourse.bass_utils import run_bass_kernel_spmd

    nc = _get_graph()
    in_maps = _make_in_maps(queries, keys, values, scale_weights, frequency_filter)
    res = run_bass_kernel_spmd(nc, in_maps, core_ids=list(range(B)))
    out = np.stack(
        [np.asarray(res.results[b]["out"]).reshape(L, H, E) for b in range(B)]
    )
    return out.astype(np.float32)


# revision 19
# speedup vs baseline: 1.1842x; 1.1842x over previous
"""Trainium2 Bass kernel for nn_AdaptiveAutoCorrelation (8-core data-parallel).

v4 — from the 253us v3:
  * q/k staged to HBM as fp16 host-side (half the input DMA, no cast op);
    LN runs all-fp16 (reduces emit fp16 stats under allow_low_precision,
    normalize is all-fp16 TT) to hit DVE 2x modes.
  * Output staged fp16 (host casts back to f32).
  * Orphan (Nyquist-only) f-tiles computed by a (-1)^p signed reduce +
    1-column matmul instead of two full DFT chains (-18k PE cycles).
  * Delay aggregation split: group 0 on DVE (fp16 stt MAC), group 1 on
    the PE (nw_k*I stationaries) so the 50%-throttled PE isn't the tail
    bottleneck.
"""
import math

import numpy as np

L = 1536
H, E = 8, 64
R = H * E  # 512
B = 8
NT = L // 128  # 12 l-tiles
SCALES = [1, 2, 4]
KT = [12, 6, 3]  # contraction tiles per scale (pooled-first)
FBINS = [L // s // 2 + 1 for s in SCALES]  # [769, 385, 193]
FT = [(f + 127) // 128 for f in FBINS]  # f-tiles per re/im block: [7, 4, 2]
NFT = 2 * sum(FT)  # 26 total f-tiles
TOPK = int(math.log(L))  # 7
LN_EPS = 1e-5
GPK = 12  # tiles packed per gather row (12KB rows)
NGRP = NT // GPK  # 2 gathers per delay
NW = 2 * L - 128 * (GPK - 1)  # 2688 rows in the sliding-window gather buffer
MC_SHIFT = 2.0 ** -14  # undo host-side M prescale (exact power of two)
MAG_EPS2 = 1e-6  # |kf|^2 floor: keeps rs finite for zero-padded bins
LN_CHUNKS = [(0, 4), (4, 8), (8, 12)]

# global ftile index bases (for S / M layout): per scale, re tiles then im
_FT_BASE = []
_acc = 0
for _s in range(len(SCALES)):
    _FT_BASE.append((_acc, _acc + FT[_s]))
    _acc += 2 * FT[_s]

_CACHE = {}


def _build_constants(scale_weights, frequency_filter):
    """D chains per scale [2*nf,128,nkt*128] fp16, M [NFT,128,L] fp16."""
    f_sig = 1.0 / (1.0 + np.exp(-np.float64(frequency_filter[0])))
    sw = np.asarray(scale_weights[: len(SCALES)], np.float64)
    w = np.exp(sw - sw.max())
    w = w / w.sum()

    d_chains = []
    M = np.zeros((NFT * 128, L), np.float64)
    for si, s in enumerate(SCALES):
        Ls = L // s
        F = FBINS[si]
        nf = FT[si]
        nkt = KT[si]
        t = np.arange(Ls)[:, None]
        f = np.arange(F)[None, :]
        ang = 2.0 * np.pi * t * f / Ls
        Dre = np.zeros((Ls, nf * 128))
        Dim = np.zeros((Ls, nf * 128))
        Dre[:, :F] = np.cos(ang)
        Dim[:, :F] = -np.sin(ang)
        # chain layout: [lf, p, kt*128 + fc] = blk[kt*128 + p, j*128 + fc]
        cr = Dre.reshape(nkt, 128, nf, 128).transpose(2, 1, 0, 3)
        ci = Dim.reshape(nkt, 128, nf, 128).transpose(2, 1, 0, 3)
        ch = np.concatenate([cr, ci], axis=0).reshape(2 * nf, 128, nkt * 128)
        d_chains.append(np.ascontiguousarray(ch.astype(np.float16)))

        reb, imb = _FT_BASE[si]
        tt = np.arange(Ls)[None, :]
        cf = np.where((f.T == 0) | (f.T == F - 1), 1.0, 2.0)
        ang2 = 2.0 * np.pi * f.T * tt / Ls
        Mre = cf * np.cos(ang2) / Ls  # [F, Ls]
        Mim = -cf * np.sin(ang2) / Ls
        if Ls != L:
            P = np.zeros((Ls, L))
            co = np.clip((np.arange(L) + 0.5) * (Ls / L) - 0.5, 0, Ls - 1)
            lo = np.floor(co).astype(int)
            hi = np.minimum(lo + 1, Ls - 1)
            fr = co - lo
            P[lo, np.arange(L)] += 1 - fr
            P[hi, np.arange(L)] += fr
            Mre = Mre @ P
            Mim = Mim @ P
        scale = w[si] * f_sig / R * 16384.0  # 2^14 prescale for fp16 range
        M[reb * 128 : reb * 128 + F] = Mre * scale
        M[imb * 128 : imb * 128 + F] = Mim * scale

    M_t = np.ascontiguousarray(M.reshape(NFT, 128, L).astype(np.float16))
    # pool-by-2 packing matrices: P2a -> out cols [0,64), P2b -> [64,128)
    P2 = np.zeros((2, 128, 128), np.float16)
    for t_ in range(128):
        P2[0, t_, t_ // 2] = 0.5
        P2[1, t_, 64 + t_ // 2] = 0.5
    I128 = np.eye(128, dtype=np.float16)
    sgn = ((-1.0) ** np.arange(128)).astype(np.float16).reshape(128, 1)
    return d_chains, M_t, P2, I128, sgn


def _build_graph():
    import concourse.bacc as bacc
    import concourse.bass as bass
    import concourse.mybir as mybir
    import concourse.tile as tile

    AF = mybir.ActivationFunctionType
    OP = mybir.AluOpType
    f32 = mybir.dt.float32
    f16 = mybir.dt.float16
    u32 = mybir.dt.uint32

    nc = bacc.Bacc("TRN2", debug=False)
    q_d = nc.dram_tensor("q", [NT, 128, R], f16, kind="ExternalInput")
    k_d = nc.dram_tensor("k", [NT, 128, R], f16, kind="ExternalInput")
    vw_d = nc.dram_tensor("vw", [NW, GPK * R], f16, kind="ExternalInput")
    d_ds = [
        nc.dram_tensor(
            f"dmat{si}", [2 * FT[si], 128, KT[si] * 128], f16,
            kind="ExternalInput",
        )
        for si in range(len(SCALES))
    ]
    m_d = nc.dram_tensor("mmat", [NFT, 128, L], f16, kind="ExternalInput")
    p_d = nc.dram_tensor("pmat", [2, 128, 128], f16, kind="ExternalInput")
    i_d = nc.dram_tensor("imat", [128, 128], f16, kind="ExternalInput")
    g_d = nc.dram_tensor("sgn", [128, 1], f16, kind="ExternalInput")
    o_d = nc.dram_tensor("out", [NT, 128, R], f16, kind="ExternalOutput")

    with tile.TileContext(nc) as tc:
        with (
            tc.tile_pool(name="qk", bufs=1) as qkpool,
            tc.tile_pool(name="small", bufs=1) as spool,
            tc.tile_pool(name="lnraw", bufs=3) as rpool,
            tc.tile_pool(name="lnstat", bufs=2) as stpool,
        ):
            xn = {}
            for name in ("q", "k"):
                xn[(name, 0)] = qkpool.tile(
                    [128, NT, R], f16, tag=f"{name}mega", name=f"{name}mega"
                )
                for si, nkt in ((1, 6), (2, 3)):
                    xn[(name, si)] = qkpool.tile(
                        [128, nkt, R], f16, tag=f"{name}p{si}",
                        name=f"{name}p{si}",
                    )

            # raw chunk loads first so transfers start ASAP
            raws = {}
            for c0, c1 in LN_CHUNKS:
                for name, src in (("q", q_d), ("k", k_d)):
                    raw = rpool.tile([128, c1 - c0, R], f16, tag=f"raw{name}")
                    nc.sync.dma_start(
                        raw[:], src.ap()[c0:c1].rearrange("t p r -> p t r")
                    )
                    raws[(name, c0)] = raw

            eps_ln = spool.tile([128, 1], f32, tag="eps_ln")
            nc.vector.memset(eps_ln[:], LN_EPS)
            eps_mag = spool.tile([128, 1], f32, tag="eps_mag")
            nc.vector.memset(eps_mag[:], MAG_EPS2)
            p2a = spool.tile([128, 128], f16, tag="p2a")
            p2b = spool.tile([128, 128], f16, tag="p2b")
            imat = spool.tile([128, 128], f16, tag="imat")
            sgn = spool.tile([128, 1], f16, tag="sgn")
            nc.sync.dma_start(p2a[:], p_d.ap()[0])
            nc.sync.dma_start(p2b[:], p_d.ap()[1])
            nc.sync.dma_start(imat[:], i_d.ap())
            nc.sync.dma_start(sgn[:], g_d.ap())
            iota2 = spool.tile([128, NGRP], u32, tag="iota2")
            for g in range(NGRP):
                nc.gpsimd.iota(
                    iota2[:, g : g + 1], pattern=[[0, 1]],
                    base=128 * GPK * g, channel_multiplier=1,
                )

            # ---- chunked pipelined layernorm (all fp16) ----
            for c0, c1 in LN_CHUNKS:
                ntc = c1 - c0
                A = ntc * H
                for name in ("q", "k"):
                    raw = raws[(name, c0)]
                    x3 = raw[:].rearrange("p t (h e) -> p (t h) e", e=E)
                    sq = rpool.tile([128, ntc, R], f16, tag=f"sq{name}")
                    nc.scalar.activation(sq[:], raw[:], AF.Square)
                    st1 = stpool.tile([128, A], f16, tag=f"st1{name}")
                    st2 = stpool.tile([128, A], f16, tag=f"st2{name}")
                    with nc.allow_low_precision("LN stats fp16 ok"):
                        nc.vector.tensor_reduce(
                            st1[:], x3, mybir.AxisListType.X, OP.add
                        )
                        nc.vector.tensor_reduce(
                            st2[:],
                            sq[:].rearrange("p t (h e) -> p (t h) e", e=E),
                            mybir.AxisListType.X, OP.add,
                        )
                    mean = stpool.tile([128, A], f32, tag=f"mn{name}")
                    nc.vector.tensor_scalar_mul(mean[:], st1[:], 1.0 / E)
                    m2 = stpool.tile([128, A], f32, tag=f"m2{name}")
                    nc.vector.tensor_mul(m2[:], mean[:], mean[:])
                    var = stpool.tile([128, A], f32, tag=f"vr{name}")
                    nc.vector.scalar_tensor_tensor(
                        var[:], st2[:], 1.0 / E, m2[:],
                        op0=OP.mult, op1=OP.subtract,
                    )
                    std = stpool.tile([128, A], f32, tag=f"sd{name}")
                    nc.scalar.activation(
                        std[:], var[:], AF.Sqrt, bias=eps_ln[:]
                    )
                    rstd = stpool.tile([128, A], f32, tag=f"rs{name}")
                    nc.vector.reciprocal(rstd[:], std[:])
                    rs16 = stpool.tile([128, A], f16, tag=f"rsh{name}")
                    nc.vector.tensor_copy(rs16[:], rstd[:])
                    sr16 = stpool.tile([128, A], f16, tag=f"srh{name}")
                    nc.vector.tensor_mul(sr16[:], mean[:], rstd[:])
                    # xhat = x*rstd - mean*rstd  (all-fp16, in-place 2nd op)
                    eng = nc.vector if name == "q" else nc.gpsimd
                    mg3 = xn[(name, 0)][:, c0:c1, :].rearrange(
                        "p t (h e) -> p (t h) e", e=E
                    )
                    rs3 = rs16[:].rearrange("p (a o) -> p a o", o=1)
                    x3b, rs_b = bass.broadcast_tensor_aps(x3, rs3)
                    eng.tensor_tensor(mg3, x3b, rs_b, OP.mult)
                    sr3 = sr16[:].rearrange("p (a o) -> p a o", o=1)
                    mg3b, sr_b = bass.broadcast_tensor_aps(mg3, sr3)
                    eng.tensor_tensor(mg3, mg3b, sr_b, OP.subtract)

            # ---- DFT + spectral + lagged irfft (mean_corr) ----
            S16 = spool.tile([128, 32], f16, tag="s16")
            nc.vector.memset(S16[:], 0.0)
            with (
                tc.tile_pool(name="psum", bufs=5, space="PSUM") as pp,
                tc.tile_pool(name="mcpsum", bufs=1, space="PSUM") as mcp,
                tc.tile_pool(name="dstream", bufs=4) as dpool,
                tc.tile_pool(name="mstream", bufs=2) as mpool,
                tc.tile_pool(name="spec", bufs=2) as scp,
            ):
                mc_ps = [
                    mcp.tile([1, 512], f32, tag=f"mc{nt}", name=f"mc{nt}")
                    for nt in range(3)
                ]

                # pools first: the PE runs them inside the LN-wait window
                for name in ("q", "k"):
                    for si, nkt in ((1, 6), (2, 3)):
                        srcm = xn[(name, si - 1)]
                        dst = xn[(name, si)]
                        for j2 in range(nkt):
                            ps = pp.tile(
                                [128, 512], f32, tag="dftps", name="poolps"
                            )
                            nc.tensor.matmul(
                                ps[:], p2a[:], srcm[:, 2 * j2, :],
                                start=True, stop=False,
                            )
                            nc.tensor.matmul(
                                ps[:], p2b[:], srcm[:, 2 * j2 + 1, :],
                                start=False, stop=True,
                            )
                            nc.scalar.activation(dst[:, j2, :], ps[:], AF.Copy)

                def is_orphan(si2, j2):
                    # last f-tile of scales 1,2 holds a single (Nyquist) bin
                    # whose imaginary part is exactly zero
                    return si2 < 2 and j2 == FT[si2] - 1

                # big scale-1 pair last: its PE chains hide the mc tail
                pair_order = (
                    [(0, j) for j in (0, 1, 2, 3, 4, 6)]
                    + [(1, j) for j in range(FT[1])]
                    + [(2, j) for j in range(FT[2])]
                    + [(0, 5)]
                )
                pair_list = []
                for si, j in pair_order:
                    reb, imb = _FT_BASE[si]
                    pair_list.append((si, j, reb + j, imb + j))
                n_pairs = len(pair_list)

                def emit_mc(pi2, first_mm):
                    si2, j2, ftr2, fti2 = pair_list[pi2]
                    fts = (ftr2,) if is_orphan(si2, j2) else (ftr2, fti2)
                    for ft in fts:
                        mtile = mpool.tile([128, L], f16, tag="mtile")
                        nc.sync.dma_start(mtile[:], m_d.ap()[ft])
                        for nt in range(3):
                            nc.tensor.matmul(
                                mc_ps[nt][:], S16[:, ft : ft + 1],
                                mtile[:, nt * 512 : (nt + 1) * 512],
                                start=first_mm,
                                stop=(
                                    pi2 == n_pairs - 1 and ft == fts[-1]
                                    and nt == 2
                                ),
                                skip_group_check=True,
                            )
                        first_mm = False
                    return first_mm

                def emit_orphan(si, ftr):
                    # single Nyquist bin: X[Ny] = sum_p sgn[p] sum_t x[p,t,:]
                    xq = xn[("q", si)]
                    xk = xn[("k", si)]
                    nkt = KT[si]
                    ys = {}
                    for nm, xm in (("q", xq), ("k", xk)):
                        y = scp.tile([128, 512], f16, tag=f"oy{nm}")
                        with nc.allow_low_precision("orphan fold fp16"):
                            nc.vector.tensor_reduce(
                                y[:],
                                xm[:].rearrange("p t r -> p r t"),
                                mybir.AxisListType.X, OP.add,
                            )
                        ps = pp.tile([128, 512], f32, tag="dftps", name="orps")
                        nc.tensor.matmul(
                            ps[0:1, :], sgn[:], y[:], start=True, stop=True
                        )
                        sb = scp.tile([1, 512], f32, tag=f"oyS{nm}")
                        nc.scalar.activation(sb[:], ps[0:1, :], AF.Copy)
                        ys[nm] = sb
                    mag = scp.tile([1, 512], f32, tag="omag")
                    nc.vector.scalar_tensor_tensor(
                        mag[:], ys["k"][:], 0.0, ys["k"][:], op0=OP.bypass,
                        op1=OP.mult,
                    )
                    smag = scp.tile([1, 512], f32, tag="osmag")
                    nc.scalar.activation(
                        smag[:], mag[:], AF.Sqrt, bias=eps_mag[0:1, 0:1]
                    )
                    rs = scp.tile([1, 512], f32, tag="ors")
                    nc.vector.reciprocal_approx_fast(rs[:], smag[:])
                    khr = scp.tile([1, 512], f32, tag="okhr")
                    nc.vector.tensor_mul(khr[:], ys["k"][:], rs[:])
                    scr = scp.tile([1, 512], f32, tag="oscr")
                    a1 = scp.tile([1, 1], f32, tag="oa1")
                    nc.vector.scalar_tensor_tensor(
                        scr[:], ys["q"][:], 0.0, khr[:], op0=OP.bypass,
                        op1=OP.mult, accum_out=a1[:],
                    )
                    nc.vector.tensor_copy(S16[0:1, ftr : ftr + 1], a1[:])

                MC_LAG = 2
                first_mm = True
                for pi, (si, j, ftr, fti) in enumerate(pair_list):
                    nkt = KT[si]
                    qx = xn[("q", si)]
                    kx = xn[("k", si)]
                    if is_orphan(si, j):
                        emit_orphan(si, ftr)
                        if pi >= MC_LAG:
                            first_mm = emit_mc(pi - MC_LAG, first_mm)
                        continue
                    psl = {}
                    # load each D tile once; q chain then k chain share it
                    for li, lf in enumerate((j, FT[si] + j)):
                        part = "re" if li == 0 else "im"
                        dch = dpool.tile([128, nkt, 128], f16, tag=f"d{si}")
                        nc.sync.dma_start(
                            dch[:].rearrange("p a b -> p (a b)"),
                            d_ds[si].ap()[lf],
                        )
                        for nm, xm in ((f"q{part}", qx), (f"k{part}", kx)):
                            ps = pp.tile(
                                [128, 512], f32, tag="dftps", name=f"ps{nm}"
                            )
                            for kt in range(nkt):
                                nc.tensor.matmul(
                                    ps[:], dch[:, kt, :], xm[:, kt, :],
                                    start=(kt == 0), stop=(kt == nkt - 1),
                                )
                            sb = scp.tile([128, 512], f16, tag=f"{nm}S")
                            nc.scalar.activation(sb[:], ps[:], AF.Copy)
                            psl[nm] = sb
                    # mc matmuls, lagged so the PE never waits on spectral
                    if pi >= MC_LAG:
                        first_mm = emit_mc(pi - MC_LAG, first_mm)
                    qreS, kreS = psl["qre"], psl["kre"]
                    qimS, kimS = psl["qim"], psl["kim"]
                    sq1 = scp.tile([128, 512], f16, tag="sq1")
                    nc.scalar.activation(sq1[:], kreS[:], AF.Square)
                    sq2 = scp.tile([128, 512], f16, tag="sq2")
                    nc.vector.scalar_tensor_tensor(
                        sq2[:], kimS[:], 0.0, kimS[:], op0=OP.bypass,
                        op1=OP.mult,
                    )
                    mag2 = scp.tile([128, 512], f16, tag="mag2")
                    nc.vector.tensor_add(mag2[:], sq1[:], sq2[:])
                    mag = scp.tile([128, 512], f32, tag="mag")
                    nc.scalar.activation(
                        mag[:], mag2[:], AF.Sqrt, bias=eps_mag[:, 0:1]
                    )
                    rs = scp.tile([128, 512], f32, tag="rs")
                    nc.vector.reciprocal_approx_fast(rs[:], mag[:])
                    khr = scp.tile([128, 512], f16, tag="khr")
                    khi = scp.tile([128, 512], f16, tag="khi")
                    nc.vector.tensor_mul(khr[:], kreS[:], rs[:])
                    nc.vector.tensor_mul(khi[:], kimS[:], rs[:])
                    scr = scp.tile([128, 512], f16, tag="scr")
                    scr2 = scp.tile([128, 512], f16, tag="scr2")
                    a1 = scp.tile([128, 1], f32, tag="a1")
                    a2 = scp.tile([128, 1], f32, tag="a2")
                    a3 = scp.tile([128, 1], f32, tag="a3")
                    a4 = scp.tile([128, 1], f32, tag="a4")
                    nc.vector.scalar_tensor_tensor(
                        scr[:], qreS[:], 0.0, khr[:], op0=OP.bypass,
                        op1=OP.mult, accum_out=a1[:],
                    )
                    nc.vector.scalar_tensor_tensor(
                        scr2[:], qimS[:], 0.0, khi[:], op0=OP.bypass,
                        op1=OP.mult, accum_out=a2[:],
                    )
                    nc.vector.tensor_add(S16[:, ftr : ftr + 1], a1[:], a2[:])
                    nc.vector.scalar_tensor_tensor(
                        scr[:], qimS[:], 0.0, khr[:], op0=OP.bypass,
                        op1=OP.mult, accum_out=a3[:],
                    )
                    nc.vector.scalar_tensor_tensor(
                        scr2[:], qreS[:], 0.0, khi[:], op0=OP.bypass,
                        op1=OP.mult, accum_out=a4[:],
                    )
                    nc.vector.tensor_sub(S16[:, fti : fti + 1], a3[:], a4[:])
                for pi in range(n_pairs - MC_LAG, n_pairs):
                    first_mm = emit_mc(pi, first_mm)

                mc_row = spool.tile([1, L], f32, tag="mcrow")
                for nt in range(3):
                    nc.vector.tensor_scalar_mul(
                        mc_row[:, nt * 512 : (nt + 1) * 512], mc_ps[nt][:],
                        MC_SHIFT,
                    )

            # ---- top-7 + softmax ----
            mc8 = spool.tile([1, 8], f32, tag="mc8")
            mcidx = spool.tile([1, 8], u32, tag="mcidx")
            nc.vector.max(mc8[:], mc_row[:])
            nc.vector.max_index(mcidx[:], mc8[:], mc_row[:])
            negmax = spool.tile([1, 1], f32, tag="negmax")
            nc.vector.tensor_scalar_mul(negmax[:], mc8[:, 0:1], -1.0)
            e7 = spool.tile([1, TOPK], f32, tag="e7")
            nc.scalar.activation(e7[:], mc8[:, 0:TOPK], AF.Exp, bias=negmax[:])
            ssum = spool.tile([1, 1], f32, tag="ssum")
            nc.vector.tensor_reduce(ssum[:], e7[:], mybir.AxisListType.X, OP.add)
            rsum = spool.tile([1, 1], f32, tag="rsum")
            nc.vector.reciprocal(rsum[:], ssum[:])
            nw = spool.tile([1, TOPK], f32, tag="nw")
            nc.vector.tensor_scalar_mul(nw[:], e7[:], rsum[:, 0:1])
            nw128 = spool.tile([128, TOPK], f32, tag="nw128")
            nc.gpsimd.partition_broadcast(nw128[:], nw[:])
            d128a = spool.tile([128, TOPK], u32, tag="d128a")
            nc.gpsimd.partition_broadcast(d128a[:], mcidx[:, 0:TOPK])
            # all 2*7 gather indices in one DVE op
            idx_all = spool.tile([128, NGRP, TOPK], u32, tag="idxall")
            ii = iota2[:].rearrange("p (g o) -> p g o", o=1)
            dd = d128a[:].rearrange("p (o k) -> p o k", o=1)
            iib, ddb = bass.broadcast_tensor_aps(ii, dd)
            nc.vector.tensor_tensor(idx_all[:], iib, ddb, OP.add)
            # weighted identity stationaries for the PE half of the MAC
            wI = []
            for kk in range(TOPK):
                wt = spool.tile([128, 128], f16, tag=f"wI{kk}", name=f"wI{kk}")
                nc.vector.tensor_scalar_mul(wt[:], imat[:], nw128[:, kk : kk + 1])
                wI.append(wt)

            # ---- gather (1 op per delay) + MAC split DVE/PE per slot ----
            HG = GPK // 2  # l-tiles per engine half
            with (
                tc.tile_pool(name="gather", bufs=3) as gpool,
                tc.tile_pool(name="gpsum", bufs=6, space="PSUM") as gpp,
            ):
                acc = gpool.tile([128, HG, R], f32, tag="acc", bufs=1)
                acc16 = gpool.tile([128, NT, R], f16, tag="acc16", bufs=1)
                gps = [
                    gpp.tile([128, 512], f32, tag="gps", name=f"gps{c}")
                    for c in range(HG)
                ]
                for kk in range(TOPK):
                    slot = gpool.tile([128, GPK * R], f16, tag="slot", bufs=3)
                    nc.gpsimd.indirect_dma_start(
                        out=slot[:],
                        out_offset=None,
                        in_=vw_d.ap(),
                        in_offset=bass.IndirectOffsetOnAxis(
                            ap=idx_all[:, 0, kk : kk + 1], axis=0
                        ),
                    )
                    av = acc[:].rearrange("p t r -> p (t r)")
                    half = slot[:, 0 : HG * R]
                    if kk == 0:
                        nc.vector.tensor_scalar_mul(av, half, nw128[:, 0:1])
                    else:
                        nc.vector.scalar_tensor_tensor(
                            av, half, nw128[:, kk : kk + 1], av,
                            op0=OP.mult, op1=OP.add,
                        )
                    for c in range(HG):
                        nc.tensor.matmul(
                            gps[c][:], wI[kk][:],
                            slot[:, (HG + c) * R : (HG + c + 1) * R],
                            start=(kk == 0), stop=(kk == TOPK - 1),
                        )
                    if kk == TOPK - 1:
                        for c in range(HG):
                            nc.scalar.activation(
                                acc16[:, c, :], acc[:, c, :], AF.Copy
                            )
                            nc.sync.dma_start(o_d.ap()[c], acc16[:, c, :])
                        for c in range(HG):
                            nc.scalar.activation(
                                acc16[:, HG + c, :], gps[c][:], AF.Copy
                            )
                            nc.sync.dma_start(
                                o_d.ap()[HG + c], acc16[:, HG + c, :]
                            )

    nc.compile()
    return nc


def _get_graph():
    if "nc" not in _CACHE:
        _CACHE["nc"] = _build_graph()
    return _CACHE["nc"]


def _make_in_maps(queries, keys, values, scale_weights, frequency_filter):
    d_chains, M_t, P2, I128, sgn = _build_constants(
        np.asarray(scale_weights, np.float64),
        np.asarray(frequency_filter, np.float64),
    )
    q = np.asarray(queries, np.float32).reshape(B, NT, 128, R).astype(np.float16)
    k = np.asarray(keys, np.float32).reshape(B, NT, 128, R).astype(np.float16)
    v = np.asarray(values, np.float32).reshape(B, L, R)
    vv = np.concatenate([v, v], axis=1).astype(np.float16)  # [B, 2L, R]
    # sliding-window buffer: vw[b, i, c, :] = vv[b, i + 128*c, :], c < GPK
    st = vv.strides
    vw = np.lib.stride_tricks.as_strided(
        vv, shape=(B, NW, GPK, R), strides=(st[0], st[1], 128 * st[1], st[2])
    )
    in_maps = []
    for b in range(B):
        m = {
            "q": np.ascontiguousarray(q[b]),
            "k": np.ascontiguousarray(k[b]),
            "vw": np.ascontiguousarray(vw[b]).reshape(NW, GPK * R),
            "mmat": M_t,
        }
        m["pmat"] = P2
        m["imat"] = I128
        m["sgn"] = sgn
        for si in range(len(SCALES)):
            m[f"dmat{si}"] = d_chains[si]
        in_maps.append(m)
    return in_maps


def kernel(queries, keys, values, scale_weights, frequency_filter, attn_mask=None):
    from concourse.bass_utils import run_bass_kernel_spmd

    nc = _get_graph()
    in_maps = _make_in_maps(queries, keys, values, scale_weights, frequency_filter)
    res = run_bass_kernel_spmd(nc, in_maps, core_ids=list(range(B)))
    out = np.stack(
        [np.asarray(res.results[b]["out"]).reshape(L, H, E) for b in range(B)]
    )
    return out.astype(np.float32)
